# revision 18
# baseline (speedup 1.0000x reference)
"""Masked cross-attention (EpipolarCrossAttention) on 8 Trainium2 NeuronCores.

Strategy: data-parallel over batch B=8 (one batch per core). Per core:
  - qT = Wq'^T xT (Wq pre-scaled by SCALE*log2e), kT = Wk^T ctxT, v = ctx Wv
  - scores sT[k, q] = kT^T qT per head into PSUM (so exp reads PSUM directly
    and the PV matmul takes p in [k, q] orientation = scores orientation)
  - exp2: ex = 2^sT, split between ACT (activation Exp with scale=ln2) and
    DVE (tensor_tensor pow with a const-2 tile); mask applied as a bf16
    multiply split between GPSIMD (SBUF-only engine) and DVE (2x mode)
  - PV FLIPPED: ao[q, d] = p^T v via lhsT=p-slice (stationary [k,128q]),
    rhs=v [k, 64]; den[q] via a ones-column matmul (N=1). Small out free-size
    (64+1 vs 1024) halves PE stream cycles vs the [d, q] orientation.
    PV for head u-1 is co-scheduled per key-tile with head u's score rounds,
    so a single p buffer recycles per key-tile slot. All 8 q-tiles of a
    head accumulate into one persistent PSUM bank; sub-bank groups rely on
    the PSUM zero-region auto-zeroing (start only on the first group).
  - division by den: per-partition tensor_scalar divide -> aoS bf16
  - transpose aoS [q, 2*64] -> aoT [128, q] per head-pair via is_transpose
    matmul with a host-supplied identity, then out = aoT^T @ Wo (+bo on host)

Host prep (not HW time): concat register tokens into context, pad keys to
2176 and channels to 384, transpose x/ctx, mask -> bf16 transposed, +bias.
"""

import math

import numpy as np
import ml_dtypes

try:
    import concourse.bass as bass  # noqa: F401
except ImportError:  # pragma: no cover
    import sys

    sys.path.insert(0, "/opt/trn_rl_repo")
    import concourse.bass as bass  # noqa: F401

import concourse.tile as tile
from concourse import bacc, mybir
from concourse.bass_utils import run_bass_kernel_spmd

BF = ml_dtypes.bfloat16
B, L1, L2, C = 8, 2048, 2048, 320
H, D = 8, 64
NREG = 4
INNER = H * D  # 512
SCALE = D ** -0.5
LK = NREG + L2  # 2052
NKT = 17
LKP = NKT * 128  # 2176
CP = 384  # padded C (3 x 128)
NCC = 3
QB = 1024
NQB = L1 // QB  # 2
NQT = QB // 128  # 8 q-tiles per q-block
f32 = mybir.dt.float32
bf16 = mybir.dt.bfloat16
Alu = mybir.AluOpType
Act = mybir.ActivationFunctionType
LN2 = math.log(2.0)

_CACHE = {}


def _build():
    nc = bacc.Bacc(None, target_bir_lowering=False)
    dp = nc.declare_dram_parameter
    xT_d = dp("xT", [CP, L1], bf16, isOutput=False)
    ctxT_d = dp("ctxT", [CP, LKP], bf16, isOutput=False)
    maskT_d = dp("maskT", [LKP, L1], bf16, isOutput=False)
    wq_d = dp("wq", [CP, INNER], bf16, isOutput=False)
    wk_d = dp("wk", [CP, INNER], bf16, isOutput=False)
    wv_d = dp("wv", [CP, INNER], bf16, isOutput=False)
    wo_d = dp("wo", [INNER, C], bf16, isOutput=False)
    ident_d = dp("ident", [128, 128], bf16, isOutput=False)
    out_d = dp("out", [L1, C], f32, isOutput=True)

    with tile.TileContext(nc) as tc:
        with tc.tile_pool(name="const", bufs=1) as const, \
             tc.tile_pool(name="big", bufs=1) as big, \
             tc.tile_pool(name="work", bufs=2) as work, \
             tc.tile_pool(name="psc", bufs=2, space="PSUM") as psc, \
             tc.tile_pool(name="pper", bufs=1, space="PSUM") as pper, \
             tc.tile_pool(name="psh", bufs=1, space="PSUM") as psh:

            # ---------------- constants + staged inputs ----------------
            attn = tc.alloc_tile_pool(name="attn", bufs=1)
            p_sb = attn.tile([128, NKT, QB], bf16, name="p_sb")
            stage = tc.alloc_tile_pool(name="stage", bufs=1)
            ctxT_sb = stage.tile([128, NCC, LKP], bf16, name="ctxT_sb")
            _ctxT_r = ctxT_d[:].rearrange("(c p) n -> p c n", p=128)
            xT_sb = stage.tile([128, NCC, L1], bf16, name="xT_sb")
            _xT_r = xT_d[:].rearrange("(c p) n -> p c n", p=128)
            wk_sb = const.tile([128, NCC, INNER], bf16)
            nc.sync.dma_start(out=wk_sb, in_=wk_d[:].rearrange("(c p) n -> p c n", p=128))
            nc.sync.dma_start(out=ctxT_sb[:, :, 0:512], in_=_ctxT_r[:, :, 0:512])
            wq_sb = const.tile([128, NCC, INNER], bf16)
            nc.sync.dma_start(out=wq_sb, in_=wq_d[:].rearrange("(c p) n -> p c n", p=128))
            nc.sync.dma_start(out=xT_sb[:, :, 0:1024], in_=_xT_r[:, :, 0:1024])
            wv_sb = const.tile([128, NCC, INNER], bf16)
            nc.sync.dma_start(out=wv_sb, in_=wv_d[:].rearrange("(c p) n -> p c n", p=128))
            for _k0, _kw in ((512, 512), (1024, 512), (1536, 640)):
                nc.sync.dma_start(out=ctxT_sb[:, :, _k0:_k0 + _kw],
                                  in_=_ctxT_r[:, :, _k0:_k0 + _kw])
            nc.sync.dma_start(out=xT_sb[:, :, 1024:2048], in_=_xT_r[:, :, 1024:2048])
            wo_sb = const.tile([128, 4, C], bf16)
            nc.sync.dma_start(out=wo_sb, in_=wo_d[:].rearrange("(c p) n -> p c n", p=128))
            ident = const.tile([128, 128], bf16)
            nc.sync.dma_start(out=ident, in_=ident_d[:])

            two_c = const.tile([128, QB], bf16)
            nc.vector.memset(two_c, 2.0)

            # preload the exp table set while DMAs run
            warm = const.tile([1, 2], f32)
            nc.vector.memset(warm, 0.0)
            nc.scalar.activation(out=warm, in_=warm, func=Act.Exp)

            qT_sb = big.tile([128, 4, L1], bf16)
            kT_sb = big.tile([128, 4, LKP], bf16)
            v_sb = big.tile([128, NKT, H, D + 1], bf16)
            aoT_sb = big.tile([128, 4, L1], bf16)
            maskq = big.tile([128, NKT, QB], bf16)
            aoS = [big.tile([128, 2, D], bf16, name=f"aoS{i}") for i in range(NQT)]
            den_sb = work.tile([128, 2, NQT], f32, tag="densb", bufs=1, name="den_sb")

            # persistent PSUM accumulators (one bank each; sub-bank groups use
            # the zero-region auto-zero: start only on the very first group).
            # q-tiles 0-6 of parity i in ao_bank[i]; q-tile 7 of both parities
            # shares ao7b (each <= 2KB = one PSUM bank).
            ao_bank = [pper.tile([128, NQT - 1, D + 1], f32, name=f"aoA{i}")
                       for i in range(2)]
            ao7b = pper.tile([128, 2, D + 1], f32, name="ao7b")

            # ---------------- qkv emitters (copies alternate ACT/DVE) ----
            cp_ct = [0]

            def psum_copy(out, in_):
                cp_ct[0] += 1
                if cp_ct[0] % 2 == 0:
                    nc.vector.tensor_copy(out=out, in_=in_)
                else:
                    nc.scalar.copy(out=out, in_=in_)

            def emit_qT_round(ic, qc):
                psf = psc.tile([128, QB], f32, tag="sc", name="ps_q")
                ps = psf[:, 0:512]
                for cc in range(NCC):
                    nc.tensor.matmul(
                        ps,
                        lhsT=wq_sb[:, cc, ic * 128:(ic + 1) * 128],
                        rhs=xT_sb[:, cc, qc * 512:(qc + 1) * 512],
                        start=(cc == 0), stop=(cc == NCC - 1),
                    )
                psum_copy(qT_sb[:, ic, qc * 512:(qc + 1) * 512], ps)

            kc_slices = [(0, 512), (512, 512), (1024, 512), (1536, 512), (2048, 128)]

            def emit_kT_round(ic, ks):
                k0, kw = ks
                psf = psc.tile([128, QB], f32, tag="sc", name="ps_k")
                ps = psf[:, 0:512]
                for cc in range(NCC):
                    nc.tensor.matmul(
                        ps[:, :kw],
                        lhsT=wk_sb[:, cc, ic * 128:(ic + 1) * 128],
                        rhs=ctxT_sb[:, cc, k0:k0 + kw],
                        start=(cc == 0), stop=(cc == NCC - 1),
                    )
                psum_copy(kT_sb[:, ic, k0:k0 + kw], ps[:, :kw])

            def emit_v_round(kt):
                psf = psc.tile([128, QB], f32, tag="sc", name="ps_v")
                ps = psf[:, 0:512]
                for cc in range(NCC):
                    nc.tensor.matmul(
                        ps,
                        lhsT=ctxT_sb[:, cc, kt * 128:(kt + 1) * 128],
                        rhs=wv_sb[:, cc, :],
                        start=(cc == 0), stop=(cc == NCC - 1),
                    )
                psum_copy(v_sb[:, kt, :, 0:D],
                          ps.rearrange("p (h d) -> p h d", h=H))

            # ---------------- attention emitters ----------------
            CPW_EXP = (2, 5, 8, 11, 14)   # exp2 via DVE copy + Pool pow
            DVE_MUL = (1, 4, 7, 10, 13, 15)

            def emit_mask_dma(kt, qb):
                nc.sync.dma_start(
                    out=maskq[:, kt, :],
                    in_=maskT_d[kt * 128:(kt + 1) * 128, qb * QB:(qb + 1) * QB])

            def emit_score_round(qb, h, kt, u=99):
                """scores + exp2 + mask for one (head, key-tile)."""
                hp, bp = h // 2, (h % 2) * 64
                sc = psc.tile([128, QB], f32, tag="sc", name="sc")
                for qc in range(QB // 512):
                    nc.tensor.matmul(
                        sc[:, qc * 512:(qc + 1) * 512],
                        lhsT=kT_sb[bp:bp + 64, hp, kt * 128:(kt + 1) * 128],
                        rhs=qT_sb[bp:bp + 64, hp,
                                  qb * QB + qc * 512: qb * QB + (qc + 1) * 512],
                        start=True, stop=True,
                    )
                ex = work.tile([128, QB], bf16, tag="ex", bufs=6, name="ex")
                if kt in CPW_EXP:
                    exc = work.tile([128, QB], bf16, tag="exc", bufs=3, name="exc")
                    nc.vector.tensor_copy(out=exc, in_=sc)
                    nc.gpsimd.tensor_tensor(ex, two_c, exc, Alu.pow)
                else:
                    nc.scalar.activation(out=ex, in_=sc, func=Act.Exp, scale=LN2)

                def do_mult(kt=kt, ex=ex, u=u):
                    ms = maskq[:, kt, :]
                    if (kt in DVE_MUL) or (u == 0 and kt not in CPW_EXP):
                        nc.vector.tensor_mul(p_sb[:, kt, :], ex, ms)
                    else:
                        nc.gpsimd.tensor_mul(p_sb[:, kt, :], ex, ms)
                return do_mult

            def emit_pv_round(pu, kt):
                """PV matmuls of unit pu for one key-tile (all q-tiles).
                v column 64 is ones -> ao column 64 is the denominator."""
                h = pu % H
                ab = ao_bank[pu % 2]
                for qt in range(NQT):
                    lw = p_sb[:, kt, qt * 128:(qt + 1) * 128]
                    dst = (ab[:, qt, :] if qt < NQT - 1
                           else ao7b[:, pu % 2, :])
                    nc.tensor.matmul(
                        dst, lhsT=lw, rhs=v_sb[:, kt, h, :],
                        start=(kt == 0 and qt in (0, NQT - 1)),
                        stop=(kt == NKT - 1),
                        skip_group_check=True,
                    )

            def emit_unit_tail(pu):
                """divides for completed unit pu; returns deferred tp/proj list."""
                qb, h = pu // H, pu % H
                par = pu % 2
                nc.scalar.copy(out=den_sb[:, par, 0:NQT - 1],
                               in_=ao_bank[par][:, :, D:D + 1])
                nc.scalar.copy(out=den_sb[:, par, NQT - 1:NQT],
                               in_=ao7b[:, par, D:D + 1])
                rc = work.tile([128, NQT], f32, tag="rc", bufs=2, name="rc")
                nc.vector.reciprocal(out=rc, in_=den_sb[:, par, :])
                for qt in range(NQT):
                    src = (ao_bank[par][:, qt, 0:D] if qt < NQT - 1
                           else ao7b[:, par, 0:D])
                    nc.vector.tensor_scalar(
                        aoS[qt][:, h % 2, :], src,
                        rc[:, qt:qt + 1], None, Alu.mult)
                if h % 2 == 0:
                    return []

                def do_transpose(qt, qb=qb, h=h):
                    tp = psh.tile([128, 128], bf16, tag="sh", name="tp")
                    nc.tensor.matmul(tp, lhsT=aoS[qt][:].rearrange("p a b -> p (a b)"),
                                     rhs=ident, is_transpose=True, start=True, stop=True)
                    nc.vector.tensor_copy(
                        out=aoT_sb[:, h // 2, qb * QB + qt * 128: qb * QB + (qt + 1) * 128],
                        in_=tp)
                from functools import partial as _pt
                post = [_pt(do_transpose, qt) for qt in range(NQT)]
                if h == 7:
                    # interleave this q-block's projections after each transpose
                    merged = []
                    for qt in range(NQT):
                        merged.append(post[qt])
                        merged.append(_pt(emit_proj, qb, qt))
                    post = merged
                return post

            def emit_proj(qb, qt):
                po = psh.tile([128, C], f32, tag="sh", name="po")
                for ic in range(4):
                    nc.tensor.matmul(
                        po,
                        lhsT=aoT_sb[:, ic, qb * QB + qt * 128: qb * QB + (qt + 1) * 128],
                        rhs=wo_sb[:, ic, :],
                        start=(ic == 0), stop=(ic == 3),
                    )
                ob = work.tile([128, C], f32, tag="ob", bufs=2, name="ob")
                psum_copy(ob, po)
                nc.sync.dma_start(
                    out=out_d[qb * QB + qt * 128: qb * QB + (qt + 1) * 128, :],
                    in_=ob)

            # ---------------- software-pipelined emission ----------------
            nc.vector.memset(v_sb[:, :, :, D:D + 1], 1.0)
            # prologue: just enough qkv for unit u=0 (head 0, qb 0)
            for ks in kc_slices:
                emit_kT_round(0, ks)
            emit_qT_round(0, 0)
            emit_qT_round(0, 1)
            # mask for qb=0 (DMA engines, overlaps compute)
            for kt in range(NKT):
                emit_mask_dma(kt, 0)

            def unit(u):
                return u // H, u % H  # qb, h

            from functools import partial
            filler_sched = {
                0: [partial(emit_v_round, kt) for kt in range(NKT)],
                1: ([partial(emit_qT_round, 1, 0), partial(emit_qT_round, 1, 1)]
                    + [partial(emit_kT_round, 1, ks) for ks in kc_slices]
                    + [partial(emit_qT_round, 2, 0), partial(emit_qT_round, 2, 1)]
                    + [partial(emit_kT_round, 2, ks) for ks in kc_slices]
                    + [partial(emit_qT_round, 3, 0), partial(emit_qT_round, 3, 1)]
                    + [partial(emit_kT_round, 3, kc_slices[0])]),
                2: [partial(emit_kT_round, 3, ks) for ks in kc_slices[1:]],
                6: [partial(emit_qT_round, ic, 2) for ic in range(4)],
                7: [partial(emit_qT_round, ic, 3) for ic in range(4)],
            }

            NU = NQB * H
            pend_post = []
            for u in range(NU):
                fillers = filler_sched.get(u, [])
                qb, h = unit(u)
                for kt in range(NKT):
                    mult_fn = emit_score_round(qb, h, kt, u=u)
                    if fillers:
                        fillers.pop(0)()
                    if pend_post:
                        pend_post.pop(0)()
                    if u > 0:
                        emit_pv_round(u - 1, kt)
                    mult_fn()  # p slot write AFTER the prior unit's PV read
                    if qb == 0 and h == 7:
                        emit_mask_dma(kt, 1)  # refill slot with qb=1 data
                if u == 7:
                    stage.release()
                if u > 0:
                    assert not pend_post
                    pend_post = emit_unit_tail(u - 1)
            # tail: PV of the last unit, its tail, remaining transposes + projs
            for kt in range(NKT):
                emit_pv_round(NU - 1, kt)
                if pend_post:
                    pend_post.pop(0)()
            while pend_post:
                pend_post.pop(0)()
            for fn in emit_unit_tail(NU - 1):
                fn()
            attn.release()
    nc.compile()
    return nc


def _prep_inputs(x, context, attn_mask, Wq, Wk, Wv, Wo, bo, reg_tokens):
    """Host-side sharding/layout prep. Returns per-core input maps."""
    s = SCALE * math.log2(math.e)
    wq_p = np.zeros((CP, INNER), BF)
    wq_p[:C] = (np.asarray(Wq, np.float32) * s).astype(BF)
    wk_p = np.zeros((CP, INNER), BF)
    wk_p[:C] = np.asarray(Wk, BF)
    wv_p = np.zeros((CP, INNER), BF)
    wv_p[:C] = np.asarray(Wv, BF)
    wo_p = np.asarray(Wo, BF)
    regT = np.asarray(reg_tokens, np.float32)[0].T.astype(BF)  # [C, NREG]
    ident = np.eye(128, dtype=np.float32).astype(BF)

    in_maps = []
    for b in range(B):
        xT = np.zeros((CP, L1), BF)
        xT[:C] = np.asarray(x[b], np.float32).T.astype(BF)
        ctxT = np.zeros((CP, LKP), BF)
        ctxT[:C, :NREG] = regT
        ctxT[:C, NREG:LK] = np.asarray(context[b], np.float32).T.astype(BF)
        maskT = np.zeros((LKP, L1), BF)
        maskT[:NREG] = BF(1.0)
        maskT[NREG:LK] = np.asarray(attn_mask[b], bool).T.astype(BF)
        in_maps.append({
            "xT": xT, "ctxT": ctxT, "maskT": maskT,
            "wq": wq_p, "wk": wk_p, "wv": wv_p, "wo": wo_p,
            "ident": ident,
        })
    return in_maps


def run(inputs, **run_kwargs):
    """Build (cached), run on 8 cores, return (output, BassKernelResults)."""
    if "nc" not in _CACHE:
        _CACHE["nc"] = _build()
    nc = _CACHE["nc"]
    in_maps = _prep_inputs(**inputs)
    res = run_bass_kernel_spmd(nc, in_maps, list(range(B)), **run_kwargs)
    bo_h = np.asarray(inputs["bo"], np.float32).reshape(1, C)
    out = np.stack([np.asarray(r["out"], np.float32) + bo_h for r in res.results],
                   axis=0)
    return out, res


def kernel(**inputs):
    out, _ = run(inputs)
    return out
